# revision 27
# baseline (speedup 1.0000x reference)
"""Trainium2 Bass kernel for nn_GRUModel (segment-GRU encoder + 1-step GRU decoder).

Sharding: data-parallel over batch B: 8 cores x 16 batches each
(rows n = b_loc*64 + c, R=1024 rows/core). Weights replicated.

Layout: fully transposed. State hT is [D(partitions), rows(free)] stored
[128, KC*R] with column index kc*R + col, so fp8 DoubleRow matmul pairs view
[128, 2, 512] directly per row-half while elementwise ops span all R=1024
columns in one instruction. Big matmuls run fp8e4m3 + DoubleRow (2 k-tiles
per instruction); the K=66 embedding matmul stays bf16. Gate PSUM tiles are
[128, 1024] (two banks, one accumulation group per half) so each sigmoid /
tanh / update is a single R-wide op.

Engine split:
  ACT : sigmoids/tanh straight from PSUM (x-side + h-side accumulated into
        the same bank), c = gh_n + bhh_n bias-fold copies
  DVE : n-path mult/add, z*d, hc/h updates
  Pool: silu multiply (psum*sg), h-n subtract, x cast, some decoder adds
  PE  : all matmuls; emb bias + (x-last) folded into a K=66 matmul
        (rows 64/65 carry -rowsum(W_emb)*last and b_emb*ones).

emb for step t+1 is emitted before the gates of step t so the PE never
idles waiting for the recurrent h chain.

Decoder exploits rank structure: h-side gates once per unique row (1024),
pe-side gates once per unique (s,c) col (512, computed before the encoder),
combined per-s with broadcast views.
"""
import numpy as np
import ml_dtypes

import concourse.bass as bass
import concourse.bacc as bacc
import concourse.mybir as mybir
from concourse import tile
from concourse.bass_utils import run_bass_kernel_spmd

bf16 = ml_dtypes.bfloat16
fp8 = ml_dtypes.float8_e4m3
F32 = mybir.dt.float32
BF16 = mybir.dt.bfloat16
FP8 = mybir.dt.float8e4
AF = mybir.ActivationFunctionType
ALU = mybir.AluOpType
DR = mybir.MatmulPerfMode.DoubleRow

B, SEQ, ENC = 128, 1024, 64
D, SEG = 512, 64
SNX = SEQ // SEG          # 16
PRED = 512
SNY = PRED // SEG         # 8
NCORES = 8
BL = B // NCORES          # 16 batches per core
R = BL * ENC              # 1024 rows per core
KC = D // 128             # 4 contraction chunks
G3 = 3 * D                # 1536 gate dims
MC = G3 // 128            # 12 gate chunks

# Precision: only the x-side gate matmul runs fp8e4m3 DoubleRow (weights +
# a dedicated fp8 copy of emb); its quantization noise is per-step and does
# not compound through the recurrence. The h-side / residual matmuls and all
# elementwise state math stay bf16 -- fp8 there breaks the 2e-2 tolerance
# (measured: Wh fp8 1.2e-2, Wres fp8 2.0e-2 relative error alone).
GP = True    # use gpsimd for elementwise ops

# bias column map
BC_RZ, BC_HN, BC_XN, BC_RES = 0, 8, 12, 16
BC_RZD, BC_HND, BC_XND, BC_PRED = 20, 28, 32, 36
NB = 37

_PROGRAM = None


def _mm(nc, ps, w, w8, rhs, mc, first, last):
    """Accumulate w[:, mc-chunk].T @ rhs into the [128, 1024] psum tile ps
    (one accumulation group per 512-col bank half).
    w flat [128, MC*KC*128] (mc-major, kc inside); rhs [128, KC*R]."""
    base = mc * (KC * 128)
    if w8:
        v3 = rhs.rearrange("p (kc fn) -> p kc fn", kc=KC)
        for i in range(KC // 2):
            wv = w[:, base + 2 * i * 128: base + 2 * i * 128 + 256] \
                .rearrange("p (two j) -> p two j", two=2)
            for f in range(2):
                nc.tensor.matmul(
                    ps[:, f * 512:(f + 1) * 512], wv,
                    v3[:, 2 * i:2 * i + 2, f * 512:(f + 1) * 512],
                    start=(first and i == 0), stop=(last and i == KC // 2 - 1),
                    perf_mode=DR)
    else:
        for kc in range(KC):
            wv = w[:, base + kc * 128: base + (kc + 1) * 128]
            for f in range(2):
                nc.tensor.matmul(
                    ps[:, f * 512:(f + 1) * 512], wv,
                    rhs[:, kc * R + f * 512: kc * R + (f + 1) * 512],
                    start=(first and kc == 0), stop=(last and kc == KC - 1))


def _build_program():
    nc = bacc.Bacc("TRN2", target_bir_lowering=False, debug=False, num_devices=8)
    x_d = nc.dram_tensor("x", [BL, SEQ, ENC], F32, kind="ExternalInput")
    aux_d = nc.dram_tensor("aux", [2, R], F32, kind="ExternalInput")
    lastrow_d = nc.dram_tensor("lastrow", [1, R], F32, kind="ExternalInput")
    wemb_d = nc.dram_tensor("wemb", [66, D], BF16, kind="ExternalInput")
    wx_d = nc.dram_tensor("wx", [128, MC * KC * 128], FP8, kind="ExternalInput")
    wh_d = nc.dram_tensor("wh", [128, MC * KC * 128], BF16, kind="ExternalInput")
    wres_d = nc.dram_tensor("wres", [128, KC * KC * 128], BF16, kind="ExternalInput")
    wxd_d = nc.dram_tensor("wxd", [128, MC * KC * 128], BF16, kind="ExternalInput")
    whd_d = nc.dram_tensor("whd", [128, MC * KC * 128], BF16, kind="ExternalInput")
    wpred_d = nc.dram_tensor("wpred", [128, KC * SEG], BF16, kind="ExternalInput")
    pet_d = nc.dram_tensor("pet", [128, KC * 512], BF16, kind="ExternalInput")
    biases_d = nc.dram_tensor("biases", [128, NB], F32, kind="ExternalInput")
    o_d = nc.dram_tensor("o", [BL, PRED, ENC], F32, kind="ExternalOutput")

    gp = nc.gpsimd if GP else nc.vector
    with tile.TileContext(nc) as tc:
        with (
            tc.tile_pool(name="wp", bufs=1) as wp,
            tc.tile_pool(name="hp", bufs=2) as hp,
            tc.tile_pool(name="psum", bufs=3, space="PSUM") as pp,
            tc.tile_pool(name="psume", bufs=2, space="PSUM") as ppe,
            tc.tile_pool(name="dgp", bufs=1) as dgp,
        ):
            # ---- persistent weights ----
            def wload(name, dram, cols, dt):
                t = wp.tile([128, cols], dt, tag=name, name=name)
                nc.sync.dma_start(t[:], dram[:])
                return t

            wemb = wp.tile([66, D], BF16, tag="wemb")
            nc.sync.dma_start(wemb[:], wemb_d[:])
            wx = wload("wx", wx_d, MC * KC * 128, FP8)
            wh = wload("wh", wh_d, MC * KC * 128, BF16)
            wres = wload("wres", wres_d, KC * KC * 128, BF16)
            wxd = wload("wxd", wxd_d, MC * KC * 128, BF16)
            whd = wload("whd", whd_d, MC * KC * 128, BF16)
            wpred = wload("wpred", wpred_d, KC * SEG, BF16)
            pet = wload("pet", pet_d, KC * 512, BF16)
            bia = wp.tile([128, NB], F32, tag="bia")
            nc.sync.dma_start(bia[:], biases_d[:])
            last64 = wp.tile([64, R], F32, tag="last64")
            nc.sync.dma_start(last64[:], lastrow_d[:].partition_broadcast(64))

            hT = None   # h unused at t=0

            # ---- decoder pe-side gates (independent of h; PE idle now) ----
            gxd = dgp.tile([128, MC * 512], BF16, tag="gxd")
            pv = pet[:].rearrange("p (kc n) -> p kc n", kc=KC)
            for mcp in range(MC // 2):
                ps = pp.tile([128, R], F32, tag="ps")
                for half in range(2):
                    mc = 2 * mcp + half
                    base = mc * (KC * 128)
                    for kc in range(KC):
                        nc.tensor.matmul(
                            ps[:, half * 512:(half + 1) * 512],
                            wxd[:, base + kc * 128: base + (kc + 1) * 128],
                            pv[:, kc, :],
                            start=(kc == 0), stop=(kc == KC - 1))
                if mcp % 2 == 0:
                    nc.scalar.copy(gxd[:, mcp * R:(mcp + 1) * R], ps[:])
                else:
                    nc.vector.tensor_copy(gxd[:, mcp * R:(mcp + 1) * R], ps[:])

            with (
                tc.tile_pool(name="xs", bufs=2) as xsp,
                tc.tile_pool(name="emb", bufs=2) as embp,
                tc.tile_pool(name="gat", bufs=1) as gatp,
                tc.tile_pool(name="tmp", bufs=3) as tmpp,
            ):
                def emit_emb_dma(t):
                    """DMA + cast segment t: returns (xsb, embT empty)."""
                    xsf = xsp.tile([66, R], F32, tag="xsf", name=f"xsf{t}")
                    nc.sync.dma_start(
                        xsf[0:64, :].rearrange("k (b c) -> k b c", b=BL),
                        x_d[:, t * SEG:(t + 1) * SEG, :]
                        .rearrange("b k c -> k b c"))
                    nc.sync.dma_start(xsf[64:66, :], aux_d[:])
                    xsb = xsp.tile([66, R], BF16, tag="xsb", name=f"xsb{t}")
                    gp.tensor_copy(xsb[:], xsf[:])
                    embT = embp.tile([128, KC * R], BF16, tag="embT",
                                     name=f"emb{t}")
                    emb8 = embp.tile([128, KC * R], FP8, tag="emb8",
                                     name=f"emb8_{t}")
                    return xsb, embT, emb8

                def emit_emb_mc(xsb, embT, emb8, mc):
                    for f in range(2):
                        pse = ppe.tile([128, 512], F32, tag="pse")
                        nc.tensor.matmul(
                            pse[:], wemb[:, mc * 128:(mc + 1) * 128],
                            xsb[:, f * 512:(f + 1) * 512],
                            start=True, stop=True)
                        sg = tmpp.tile([128, 512], BF16, tag="sg")
                        nc.scalar.activation(sg[:], pse[:], AF.Sigmoid)
                        sl = slice(mc * R + f * 512, mc * R + (f + 1) * 512)
                        nc.vector.tensor_tensor(
                            embT[:, sl], pse[:], sg[:], ALU.mult)
                        gp.tensor_copy(emb8[:, sl], embT[:, sl])

                nxt = emit_emb_dma(0)
                for mc in range(KC):
                    emit_emb_mc(*nxt, mc)
                embT, emb8T = nxt[1], nxt[2]
                for t in range(SNX):
                    if t + 1 < SNX:
                        nxt = emit_emb_dma(t + 1)
                        emit_emb_mc(*nxt, 0)
                        emit_emb_mc(*nxt, 1)
                        emb_next, emb8_next = nxt[1], nxt[2]
                    else:
                        emb_next = emb8_next = None
                    rz = gatp.tile([128, 8 * R], BF16, tag="rz", name=f"rz{t}")
                    nf = gatp.tile([128, KC * R], BF16, tag="nf", name=f"nf{t}")
                    hc8 = gatp.tile([128, KC * R], BF16, tag="hc", name=f"hc{t}")
                    hT_new = hp.tile([128, KC * R], BF16, tag="h", name=f"h{t}")

                    def rz_gate(mc):
                        ps = pp.tile([128, R], F32, tag="ps")
                        _mm(nc, ps, wx, True, emb8T[:], mc, True, t == 0)
                        if t > 0:
                            _mm(nc, ps, wh, False, hT[:], mc, False, True)
                        nc.scalar.activation(
                            rz[:, mc * R:(mc + 1) * R], ps[:], AF.Sigmoid,
                            bias=bia[:, BC_RZ + mc: BC_RZ + mc + 1])

                    # -- r gates first (they gate the n-path) --
                    for mc in range(4):
                        rz_gate(mc)
                    # -- n gate: t1 = (psh + bhh_n) * r in one Pool stt --
                    for mc in range(KC):
                        rsl = rz[:, mc * R:(mc + 1) * R]
                        t1 = tmpp.tile([128, R], BF16, tag="t1")
                        if t > 0:
                            psh = pp.tile([128, R], F32, tag="ps")
                            _mm(nc, psh, wh, False, hT[:], 8 + mc, True, True)
                            nc.vector.scalar_tensor_tensor(
                                t1[:], psh[:],
                                bia[:, BC_HN + mc: BC_HN + mc + 1], rsl,
                                ALU.add, ALU.mult)
                        else:
                            nc.vector.tensor_scalar(
                                t1[:], rsl,
                                bia[:, BC_HN + mc: BC_HN + mc + 1], None,
                                ALU.mult)
                        psx = pp.tile([128, R], F32, tag="ps")
                        _mm(nc, psx, wx, True, emb8T[:], 8 + mc, True, True)
                        u = tmpp.tile([128, R], BF16, tag="u")
                        nc.vector.tensor_tensor(u[:], psx[:], t1[:], ALU.add)
                        nc.scalar.activation(
                            nf[:, mc * R:(mc + 1) * R], u[:], AF.Tanh,
                            bias=bia[:, BC_XN + mc: BC_XN + mc + 1])
                    # -- z gates --
                    for mc in range(4, 8):
                        rz_gate(mc)
                    # rest of next step's embedding (ACT sigmoids queue here,
                    # off the recurrent chain)
                    if t + 1 < SNX:
                        emit_emb_mc(*nxt, 2)
                        emit_emb_mc(*nxt, 3)
                    # -- hc = n + z*(h-n); chain alternates DVE/Pool per mc --
                    for mc in range(KC):
                        nsl = nf[:, mc * R:(mc + 1) * R]
                        zsl = rz[:, (4 + mc) * R:(5 + mc) * R]
                        csl = hc8[:, mc * R:(mc + 1) * R]
                        m = tmpp.tile([128, R], BF16, tag="m")
                        if t > 0:
                            d = tmpp.tile([128, R], BF16, tag="d")
                            gp.tensor_tensor(
                                d[:], hT[:, mc * R:(mc + 1) * R], nsl,
                                ALU.subtract)
                            gp.tensor_tensor(m[:], zsl, d[:], ALU.mult)
                            gp.tensor_tensor(csl, nsl, m[:], ALU.add)
                        else:
                            gp.tensor_tensor(m[:], zsl, nsl, ALU.mult)
                            gp.tensor_tensor(csl, nsl, m[:],
                                                    ALU.subtract)
                    # -- h_new = emb + (hc @ resW^T + res_b) --
                    for mc in range(KC):
                        psr = pp.tile([128, R], F32, tag="ps")
                        _mm(nc, psr, wres, False, hc8[:], mc, True, True)
                        nc.vector.scalar_tensor_tensor(
                            hT_new[:, mc * R:(mc + 1) * R],
                            psr[:], bia[:, BC_RES + mc: BC_RES + mc + 1],
                            embT[:, mc * R:(mc + 1) * R],
                            ALU.add, ALU.add)
                    hT = hT_new
                    embT, emb8T = emb_next, emb8_next

            # ================= decoder =================
            with (
                tc.tile_pool(name="dw", bufs=2) as dwp,
            ):
                # h-side gates for the 1024 unique rows: ghd [G3, R] bf16
                # (n-chunks get bhh_n bias folded in)
                ghd = dgp.tile([128, MC * R], BF16, tag="ghd")
                for mc in range(MC):
                    ps = pp.tile([128, R], F32, tag="ps")
                    _mm(nc, ps, whd, False, hT[:], mc, True, True)
                    dst = ghd[:, mc * R:(mc + 1) * R]
                    if mc >= 8:
                        if mc % 2 == 0:
                            nc.scalar.activation(
                                dst, ps[:], AF.Identity,
                                bias=bia[:, BC_HND + mc - 8: BC_HND + mc - 7])
                        else:
                            nc.vector.tensor_scalar(
                                dst, ps[:],
                                bia[:, BC_HND + mc - 8: BC_HND + mc - 7],
                                None, ALU.add)
                    elif mc % 2 == 0:
                        nc.scalar.copy(dst, ps[:])
                    else:
                        nc.vector.tensor_copy(dst, ps[:])

                def gxv(mc, s):   # pe-side view for fixed s: broadcast over b
                    v = gxd[:, mc * 512 + s * ENC: mc * 512 + (s + 1) * ENC]
                    return v.unsqueeze(1).to_broadcast((128, BL, ENC))

                for s in range(SNY):
                    rzd = dwp.tile([128, 8 * R], BF16, tag="rzd", name=f"rzd{s}")
                    for mc in range(8):
                        u = dwp.tile([128, R], BF16, tag="du")
                        eng = gp if mc % 2 == 1 else nc.vector
                        eng.tensor_tensor(
                            u[:].rearrange("p (b c) -> p b c", b=BL),
                            ghd[:, mc * R:(mc + 1) * R]
                            .rearrange("p (b c) -> p b c", b=BL),
                            gxv(mc, s), ALU.add)
                        nc.scalar.activation(
                            rzd[:, mc * R:(mc + 1) * R], u[:], AF.Sigmoid,
                            bias=bia[:, BC_RZD + mc: BC_RZD + mc + 1])
                    nd = dwp.tile([128, 4 * R], BF16, tag="nd", name=f"nd{s}")
                    for mc in range(4):
                        # c (=ghd_n + gbhh_n) already folded into ghd
                        t1 = dwp.tile([128, R], BF16, tag="dt1")
                        nc.vector.tensor_tensor(
                            t1[:], rzd[:, mc * R:(mc + 1) * R],
                            ghd[:, (8 + mc) * R:(9 + mc) * R], ALU.mult)
                        t2 = dwp.tile([128, R], BF16, tag="dt2")
                        eng = gp if mc == 3 else nc.vector
                        eng.tensor_tensor(
                            t2[:].rearrange("p (b c) -> p b c", b=BL),
                            t1[:].rearrange("p (b c) -> p b c", b=BL),
                            gxv(8 + mc, s), ALU.add)
                        nc.scalar.activation(
                            nd[:, mc * R:(mc + 1) * R], t2[:], AF.Tanh,
                            bias=bia[:, BC_XND + mc: BC_XND + mc + 1])
                    # hy = n + z*(h0d - n)  (h-layout [128, KC*R])
                    hy = dwp.tile([128, KC * R], BF16, tag="hy", name=f"hy{s}")
                    for mc in range(KC):
                        nsl = nd[:, mc * R:(mc + 1) * R]
                        zsl = rzd[:, (4 + mc) * R:(5 + mc) * R]
                        ysl = hy[:, mc * R:(mc + 1) * R]
                        d = dwp.tile([128, R], BF16, tag="dd")
                        gp.tensor_tensor(
                            d[:], hT[:, mc * R:(mc + 1) * R], nsl,
                            ALU.subtract)
                        m = dwp.tile([128, R], BF16, tag="dm")
                        nc.vector.tensor_tensor(m[:], zsl, d[:], ALU.mult)
                        nc.vector.tensor_tensor(ysl, nsl, m[:], ALU.add)
                    # y = hy @ predW^T + pred_b + last
                    yt = dwp.tile([64, R], F32, tag="yt")
                    hv = hy[:].rearrange("p (kc fn) -> p kc fn", kc=KC)
                    ps = pp.tile([128, R], F32, tag="ps")
                    for f in range(2):
                        for kc in range(KC):
                            nc.tensor.matmul(
                                ps[0:64, f * 512:(f + 1) * 512],
                                wpred[:, kc * SEG:(kc + 1) * SEG],
                                hv[:, kc, f * 512:(f + 1) * 512],
                                start=(kc == 0), stop=(kc == KC - 1))
                    nc.vector.scalar_tensor_tensor(
                        yt[:], ps[0:64, :], bia[0:64, BC_PRED: BC_PRED + 1],
                        last64[:], ALU.add, ALU.add)
                    # store: o[b, s*64+k, c] = yt[k, b*64 + c]
                    nc.sync.dma_start(
                        o_d[:, s * SEG:(s + 1) * SEG, :]
                        .rearrange("b k c -> k b c"),
                        yt[:].rearrange("k (b c) -> k b c", b=BL))
    nc.finalize()
    return nc


def _wpack(W, dt):
    """W [out dims, in dims] -> flat [128, MC*KC*128] stationary layout:
    [p, mc*KC*128 + kc*128 + j] = W[mc*128 + j, kc*128 + p]."""
    O, I = W.shape
    mc, kc = O // 128, I // 128
    t = W.reshape(mc, 128, kc, 128).transpose(3, 0, 2, 1)
    return np.ascontiguousarray(t.reshape(128, mc * kc * 128)).astype(dt)


def _prep_host(inputs):
    f = lambda a: np.ascontiguousarray(a, dtype=np.float32)
    W_emb = f(inputs["W_emb"])                      # (D, SEG)
    wemb = np.zeros((66, D), np.float32)
    wemb[0:64, :] = W_emb.T
    wemb[64, :] = -W_emb.sum(axis=1)
    wemb[65, :] = f(inputs["b_emb"])
    Wih, Whh = f(inputs["cell_Wih"]), f(inputs["cell_Whh"])
    bih, bhh = f(inputs["cell_bih"]), f(inputs["cell_bhh"])
    resW, resb = f(inputs["res_W"]), f(inputs["res_b"])
    gWih, gWhh = f(inputs["gru_Wih"]), f(inputs["gru_Whh"])
    gbih, gbhh = f(inputs["gru_bih"]), f(inputs["gru_bhh"])
    predW, predb = f(inputs["pred_W"]), f(inputs["pred_b"])
    pos_emb, channel_emb = f(inputs["pos_emb"]), f(inputs["channel_emb"])

    pe = np.zeros((D, SNY * ENC), np.float32)       # cols j = s*64 + c
    half = D // 2
    pe[0:half, :] = np.repeat(pos_emb.T, ENC, axis=1)          # pos[s,:] per col
    pe[half:, :] = np.tile(channel_emb.T, (1, SNY))            # ch[c,:] per col
    # pet: moving layout [128, KC*512], kc-chunks contiguous
    pet = np.ascontiguousarray(
        pe.reshape(KC, 128, SNY * ENC).transpose(1, 0, 2).reshape(128, KC * 512)
    ).astype(bf16)

    # wpred: [p, kc*64 + j] = predW[j, kc*128+p]
    wpred = np.ascontiguousarray(
        predW.T.reshape(KC, 128, SEG).transpose(1, 0, 2).reshape(128, KC * SEG)
    ).astype(bf16)

    biases = np.zeros((128, NB), np.float32)

    def put(col, vec):
        nch = max(1, len(vec) // 128)
        for i in range(nch):
            seg = vec[i * 128:(i + 1) * 128]
            biases[0:len(seg), col + i] = seg

    put(BC_RZ, (bih + bhh)[0:1024])
    put(BC_HN, bhh[1024:1536])
    put(BC_XN, bih[1024:1536])
    put(BC_RES, resb)
    put(BC_RZD, (gbih + gbhh)[0:1024])
    put(BC_HND, gbhh[1024:1536])
    put(BC_XND, gbih[1024:1536])
    put(BC_PRED, predb)

    return {
        "wemb": np.ascontiguousarray(wemb).astype(bf16),
        "wx": _wpack(Wih, fp8), "wh": _wpack(Whh, bf16),
        "wres": _wpack(resW, bf16),
        "wxd": _wpack(gWih, bf16), "whd": _wpack(gWhh, bf16),
        "wpred": wpred, "pet": pet, "biases": biases,
    }


def kernel(**inputs):
    global _PROGRAM
    if _PROGRAM is None:
        _PROGRAM = _build_program()
    nc = _PROGRAM
    shared = _prep_host(inputs)
    x = np.ascontiguousarray(inputs["x"], dtype=np.float32)
    in_maps = []
    for c in range(NCORES):
        xs = x[c * BL:(c + 1) * BL]
        m = dict(shared)
        m["x"] = xs
        last = xs[:, -1, :].reshape(1, R)
        m["lastrow"] = np.ascontiguousarray(last)
        aux = np.ones((2, R), np.float32)
        aux[0] = last
        m["aux"] = aux
        in_maps.append(m)
    res = run_bass_kernel_spmd(nc, in_maps, list(range(NCORES)))
    out = np.concatenate([res.results[c]["o"] for c in range(NCORES)], axis=0)
    return out.astype(np.float32)
